# revision 1
# baseline (speedup 1.0000x reference)
"""Single-head attention (B=2, S=2048, D=2048, fp32) on 8 trn2 NeuronCores.

Sharding: sequence-parallel. The 4096 tokens (B*S) are split 512/core; cores
0-3 hold batch 0, cores 4-7 batch 1. Each core computes K^T/V for its 512
tokens, the shards are all-gathered within each 4-core group (one group per
batch), then each core computes scores -> softmax -> attn@V -> @W_o for its
512 queries against the full 2048 keys of its batch.

All matmul operands are bf16 (1 cycle/row on the PE, same as fp32r, but half
the HBM/collective traffic and SBUF footprint); accumulation is fp32 in PSUM.

per-core phases (each 256 matmuls of K=128, M=128, N=512):
  B: KT_shard(e,t) = mm(lhsT=W_k[d,e], rhs=xt[d,t])        -> DRAM, AllGather
  C: V_shard(t,e)  = mm(lhsT=xt[d,t],  rhs=W_v[d,e])       -> DRAM, AllGather
  D: QT(e,q)       = mm(lhsT=W_q'[d,e], rhs=xt[d,q])       -> SBUF (W_q'=W_q/sqrt(D))
  E: scoresT(k,q)  = mm(lhsT=KTg[e,k], rhs=QT[e,q]); exp -> attnT in SBUF
     (transposed-score formulation: no PE transposes, no extra copies)
  F: outT(e,q)     = mm(lhsT=Vg[k,e], rhs=attnT[k,q])
  S: rowsums(q)    = 64 tiny mm(lhsT=attnT[k,q-blk], rhs=ones) -> 1 PSUM bank
     (issued after F so they never wait on the exp activations)
  G: final(q,d)    = mm(lhsT=outT[e,q], rhs=W_o[e,d]) * (1/rowsum) -> out DRAM

Weight loads are (128,1024) bf16 tiles (2KB lines, half the DMACopy count);
x^T ships as one (128, 8192) tile in a single 2MB DMA; loads round-robin the
SP/Activation HWDGE queues, stores go on SP. Matmuls issue in "j16" order
(16 consecutive same-bank chained matmuls — measured ~64ns cheaper per
boundary than alternating banks) with all 8 PSUM banks in rotation.
"""
import math
import numpy as np
import ml_dtypes

import concourse.bass as bass
import concourse.mybir as mybir
import concourse.tile as tile
from concourse import bacc
from concourse.bass_utils import run_bass_kernel_spmd

F32 = mybir.dt.float32
BF = mybir.dt.bfloat16
BFNP = np.dtype(ml_dtypes.bfloat16)

D = 2048          # d_model
B = 2
S = 2048
NCORES = 8
GS = 4            # cores per batch (collective group size)
TOK = 512         # tokens per core
P = 128
NT = D // P       # 16 tiles along d/e
QT_N = TOK // P   # 4 query tiles per core
KC_N = S // TOK   # 4 key chunks (one per source core in the group)
SGW = 1024        # weight supergroup width (free dim of one weight DMA)


def build_attn(n_iters=1, with_collective=True, dma_mode="hwdge",
               rowsums=True, psum_bufs=8, no_dma=False, order="j16",
               qsplit=False, pair_e=True, pair_c=True, **_compat):
    """Build the SPMD attention kernel. n_iters>1 wraps ALL compute phases in
    a timing loop (collectives must be disabled for that, unless
    with_collective=="loop" which runs the AllGathers inside the loop).

    dma_mode: "hwdge" round-robins all DMA over the SP/Activation HWDGE
    queues; "pool" puts streams/stores on the gpsimd SWDGE queue.
    rowsums=False skips the softmax denominator (timing experiments only).
    """
    assert n_iters == 1 or with_collective in (False, "loop")
    nc = bacc.Bacc("TRN2", target_bir_lowering=False, debug=False, num_devices=NCORES)

    xt = nc.dram_tensor("xt", [P, NT * TOK], BF, kind="ExternalInput")
    wq = nc.dram_tensor("wq", [D, D], BF, kind="ExternalInput")
    wk = nc.dram_tensor("wk", [D, D], BF, kind="ExternalInput")
    wv = nc.dram_tensor("wv", [D, D], BF, kind="ExternalInput")
    wo = nc.dram_tensor("wo", [D, D], BF, kind="ExternalInput")
    out = nc.dram_tensor("out", [TOK, D], F32, kind="ExternalOutput")

    with tile.TileContext(nc) as tc:
        with (
            tc.tile_pool(name="dram", bufs=1, space="DRAM") as dram,
            tc.tile_pool(name="big", bufs=1) as big,
            tc.tile_pool(name="wpool", bufs=32) as wpool,
            tc.tile_pool(name="epool", bufs=16 if pair_e else 32) as epool,
            tc.tile_pool(name="fpool",
                         bufs=11 if pair_c else (22 if pair_e else 24)) as fpool,
            tc.tile_pool(name="qtpool", bufs=NT) as qtpool,
            tc.tile_pool(name="evpool", bufs=4) as evpool,
            tc.tile_pool(name="oevpool", bufs=4) as oevpool,
            tc.tile_pool(name="misc", bufs=1) as misc,
            tc.tile_pool(name="ps", bufs=psum_bufs, space="PSUM") as ps,
            tc.tile_pool(name="pss", bufs=8 - psum_bufs or 1,
                         space="PSUM") as pss,
        ):
            kt_shard = dram.tile([D, TOK], BF)
            v_shard = dram.tile([TOK, D], BF)
            kt_g = dram.tile([GS * D, TOK], BF)    # [s*D + e, k_local]
            v_g = dram.tile([GS * TOK, D], BF)     # [k, e]

            # x^T lives as one (128, 16*512) tile: block dt at cols
            # [dt*TOK, (dt+1)*TOK) holds x^T[dt*128:(dt+1)*128, :] — loaded by
            # a single 2MB DMA (128 lines x 16KB) at kernel start.
            xt_big = big.tile([P, NT * TOK], BF, name="xt_big")

            def xts(dt, c0=0, c1=TOK):
                return xt_big[:, dt * TOK + c0: dt * TOK + c1]

            attnT_sb = [big.tile([P, TOK], BF, name=f"attnT{i}") for i in range(NT)]

            ones = misc.tile([P, 1], BF)
            nc.vector.memset(ones[:], 1.0)
            tot = misc.tile([P, QT_N], F32)
            recip = misc.tile([P, QT_N], F32)

            _rr = [0]
            _hwdge = [nc.sync] if qsplit else [nc.sync, nc.scalar]

            def LD(dst, src_ap):
                """Stream-load round-robin over the HWDGE load queues."""
                if no_dma:
                    return
                e = _hwdge[_rr[0] % len(_hwdge)]
                _rr[0] += 1
                e.dma_start(dst, src_ap)

            if dma_mode == "pool":
                def ST(dst, src_ap):
                    if not no_dma:
                        nc.gpsimd.dma_start(dst, src_ap)
            else:
                _st_eng = nc.scalar if qsplit else nc.sync

                def ST(dst, src_ap):
                    if not no_dma:
                        _st_eng.dma_start(dst, src_ap)

            def load_w_sg(w_dram, sg, pfx):
                """Load one (128,1024)-tile supergroup of a weight matrix."""
                ts = []
                for dt in range(NT):
                    t = wpool.tile([P, SGW], BF, tag="w", name=f"{pfx}{sg}_{dt}")
                    LD(t[:], w_dram[dt * P:(dt + 1) * P, sg * SGW:(sg + 1) * SGW])
                    ts.append(t)
                return ts

            def mm_order(n_chain=NT, n_banks=4):
                """Yield (chain_pos, bank). order="dt": banks rotate every
                instruction; order="j": half-split, 8 consecutive same-bank
                matmuls (the v0 chain16 pattern)."""
                if order == "dt":
                    for c in range(n_chain):
                        for b in range(n_banks):
                            yield c, b
                elif order == "j16":
                    for b in range(n_banks):
                        for c in range(n_chain):
                            yield c, b
                else:
                    h = n_chain // 2
                    for half in range(2):
                        for b in range(n_banks):
                            for c8 in range(h):
                                yield half * h + c8, b

            def proj_to_T(w_dram, dest_cb, pfx):
                """out[e,*] = sum_d W[d,e] * xt[d,*]; 16 chain-16 psums."""
                for sg in range(2):
                    wts = load_w_sg(w_dram, sg, pfx)
                    for quad in range(2):
                        psums = [ps.tile([P, 512], F32, tag="mm",
                                         name=f"{pfx}p{quad}{j}") for j in range(4)]
                        for dt, j in mm_order():
                            c0 = quad * 512 + j * P
                            nc.tensor.matmul(
                                psums[j][:], wts[dt][:, c0:c0 + P],
                                xts(dt),
                                start=(dt == 0), stop=(dt == NT - 1))
                        dest_cb(sg * 2 + quad, psums)

            def b_dest(eg, psums):
                if pair_e:
                    # Pair two psums into one (128,1024) ev and store as a
                    # (256,512) block. The DMA's natural linearization writes
                    # row 2p+h from ev[p, h*512:...], i.e. an e-interleaved
                    # DRAM layout — which the paired E load (256,512)->
                    # (128,1024) inverts exactly.
                    for jp in range(2):
                        et0 = eg * 4 + 2 * jp
                        ev = evpool.tile([P, 1024], BF, tag="ev", name="evb")
                        nc.scalar.copy(ev[:, 0:512], psums[2 * jp][:])
                        nc.scalar.copy(ev[:, 512:1024], psums[2 * jp + 1][:])
                        ST(kt_shard[et0 * P:(et0 + 2) * P, :], ev[:])
                    return
                for j in range(4):
                    et = eg * 4 + j
                    ev = evpool.tile([P, 512], BF, tag="ev", name="evb")
                    nc.scalar.copy(ev[:], psums[j][:])
                    ST(kt_shard[et * P:(et + 1) * P, :], ev[:])

            def phase_c():
                for sg in range(2):
                    wvs = load_w_sg(wv, sg, "cw")
                    for half in range(2):
                        psums = [ps.tile([P, 512], F32, tag="mm",
                                         name=f"pvp{tt}") for tt in range(4)]
                        for dt, tt in mm_order():
                            nc.tensor.matmul(
                                psums[tt][:],
                                xts(dt, tt * P, (tt + 1) * P),
                                wvs[dt][:, half * 512:(half + 1) * 512],
                                start=(dt == 0), stop=(dt == NT - 1))
                        ec = sg * 2 + half
                        if pair_c:
                            for tp in range(2):
                                ev = evpool.tile([P, 1024], BF, tag="ev",
                                                 name="evc")
                                nc.scalar.copy(ev[:, 0:512], psums[2 * tp][:])
                                nc.scalar.copy(ev[:, 512:1024],
                                               psums[2 * tp + 1][:])
                                ST(v_shard[tp * 256:(tp + 1) * 256,
                                           ec * 512:(ec + 1) * 512], ev[:])
                        else:
                            for tt in range(4):
                                ev = evpool.tile([P, 512], BF, tag="ev",
                                                 name="evc")
                                nc.scalar.copy(ev[:], psums[tt][:])
                                ST(v_shard[tt * P:(tt + 1) * P,
                                           ec * 512:(ec + 1) * 512], ev[:])

            def phases_defg():
                # ---- phase D: QT (tiles share slots with outT via tag)
                qt_sb = [qtpool.tile([P, TOK], BF, tag="qo", name=f"qt{i}")
                         for i in range(NT)]

                def d_dest(eg, psums):
                    for j in range(4):
                        nc.scalar.copy(qt_sb[eg * 4 + j][:], psums[j][:])
                proj_to_T(wq, d_dest, "pd")

                # ---- phase E: scoresT + exp straight into attnT
                for kc in range(KC_N):
                    kts = []
                    if pair_e:
                        for etp in range(NT // 2):
                            t = epool.tile([P, 1024], BF, tag="e",
                                           name=f"ek{kc}_{etp}")
                            LD(t[:], kt_g[kc * D + etp * 256:
                                          kc * D + (etp + 1) * 256, :])
                            kts.append(t)
                    else:
                        for et in range(NT):
                            t = epool.tile([P, 512], BF, tag="e",
                                           name=f"ek{kc}_{et}")
                            if dma_mode == "pool" and et % 2 == 1 and not no_dma:
                                nc.gpsimd.dma_start(
                                    t[:],
                                    kt_g[kc * D + et * P: kc * D + (et + 1) * P, :])
                            else:
                                LD(t[:],
                                   kt_g[kc * D + et * P: kc * D + (et + 1) * P, :])
                            kts.append(t)
                    psums = [ps.tile([P, 512], F32, tag="mm", name=f"pep{j}")
                             for j in range(4)]
                    for et, j in mm_order():
                        if pair_e:
                            lhsT = kts[et // 2][:, (et % 2) * 512 + j * P:
                                                (et % 2) * 512 + (j + 1) * P]
                        else:
                            lhsT = kts[et][:, j * P:(j + 1) * P]
                        nc.tensor.matmul(
                            psums[j][:], lhsT,
                            qt_sb[et][:],
                            start=(et == 0), stop=(et == NT - 1))
                    for j in range(4):
                        nc.scalar.activation(
                            attnT_sb[kc * 4 + j][:], psums[j][:],
                            mybir.ActivationFunctionType.Exp)

                # ---- phase F: outT (slots freed by qt after phase E)
                outT_sb = [qtpool.tile([P, TOK], BF, tag="qo", name=f"outT{i}")
                           for i in range(NT)]
                for sg in range(2):
                    vts = []
                    if pair_c:
                        for ktp in range(NT // 2):
                            t = fpool.tile([P, 2 * SGW], BF, tag="f",
                                           name=f"fv{sg}_{ktp}")
                            LD(t[:], v_g[ktp * 256:(ktp + 1) * 256,
                                         sg * SGW:(sg + 1) * SGW])
                            vts.append(t)
                    else:
                        for kt in range(NT):
                            t = fpool.tile([P, SGW], BF, tag="f",
                                           name=f"fv{sg}_{kt}")
                            if dma_mode == "pool" and not no_dma:
                                nc.gpsimd.dma_start(
                                    t[:],
                                    v_g[kt * P:(kt + 1) * P,
                                        sg * SGW:(sg + 1) * SGW])
                            else:
                                LD(t[:],
                                   v_g[kt * P:(kt + 1) * P,
                                       sg * SGW:(sg + 1) * SGW])
                            vts.append(t)
                    for quad in range(2):
                        psums = [ps.tile([P, 512], F32, tag="mm", name=f"pfp{j}")
                                 for j in range(4)]
                        for kt, j in mm_order():
                            c0 = quad * 512 + j * P
                            if pair_c:
                                lhsT = vts[kt // 2][:, (kt % 2) * SGW + c0:
                                                    (kt % 2) * SGW + c0 + P]
                            else:
                                lhsT = vts[kt][:, c0:c0 + P]
                            nc.tensor.matmul(
                                psums[j][:], lhsT,
                                attnT_sb[kt][:],
                                start=(kt == 0), stop=(kt == NT - 1))
                        for j in range(4):
                            nc.scalar.copy(outT_sb[(sg * 2 + quad) * 4 + j][:],
                                           psums[j][:])

                # ---- rowsums: 64 tiny matmuls into one PSUM bank.
                # Issued after F's matmuls so the PE never waits on the exps.
                if rowsums:
                    spool = ps if psum_bufs == 8 else pss
                    psum_s = spool.tile([P, QT_N, KC_N], F32,
                                        tag="mm" if psum_bufs == 8 else "s",
                                        name="psum_s")
                    for kc in range(KC_N):
                        for qt in range(QT_N):
                            for j in range(4):
                                nc.tensor.matmul(
                                    psum_s[:, qt, kc:kc + 1],
                                    attnT_sb[kc * 4 + j][:, qt * P:(qt + 1) * P],
                                    ones[:],
                                    start=(j == 0), stop=(j == 3))
                    for qt in range(QT_N):
                        nc.vector.reduce_sum(tot[:, qt:qt + 1], psum_s[:, qt, :],
                                             axis=mybir.AxisListType.X)
                    nc.vector.reciprocal(recip[:], tot[:])
                else:
                    nc.vector.memset(recip[:], 1.0)

                # ---- phase G: final projection + normalize
                for sg in range(2):
                    wos = load_w_sg(wo, sg, "gw")
                    for half in range(2):
                        psums = [ps.tile([P, 512], F32, tag="mm", name=f"pgp{qt}")
                                 for qt in range(4)]
                        for et, qt in mm_order():
                            nc.tensor.matmul(
                                psums[qt][:],
                                outT_sb[et][:, qt * P:(qt + 1) * P],
                                wos[et][:, half * 512:(half + 1) * 512],
                                start=(et == 0), stop=(et == NT - 1))
                        dc = sg * 2 + half
                        for qt in range(4):
                            evf = oevpool.tile([P, 512], F32, tag="of", name="evf")
                            nc.vector.tensor_scalar_mul(evf[:], psums[qt][:],
                                                        recip[:, qt:qt + 1])
                            ST(out[qt * P:(qt + 1) * P, dc * 512:(dc + 1) * 512],
                               evf[:])

            def whole_body():
                if not no_dma:
                    if dma_mode == "pool":
                        nc.gpsimd.dma_start(xt_big[:], xt[:])
                    else:
                        nc.sync.dma_start(xt_big[:], xt[:])
                proj_to_T(wk, b_dest, "pb")
                if with_collective in (True, "loop", "k"):
                    nc.gpsimd.collective_compute(
                        "AllGather", mybir.AluOpType.bypass,
                        replica_groups=[[0, 1, 2, 3], [4, 5, 6, 7]],
                        ins=[kt_shard[:].opt()], outs=[kt_g[:].opt()],
                    )
                phase_c()
                if with_collective in (True, "loop", "v"):
                    nc.gpsimd.collective_compute(
                        "AllGather", mybir.AluOpType.bypass,
                        replica_groups=[[0, 1, 2, 3], [4, 5, 6, 7]],
                        ins=[v_shard[:].opt()], outs=[v_g[:].opt()],
                    )
                phases_defg()

            if n_iters == 1:
                whole_body()
            else:
                with tc.For_i(0, n_iters, 1):
                    whole_body()

    nc.compile()
    return nc


_CACHED = {}


def _get_nc():
    if "nc" not in _CACHED:
        _CACHED["nc"] = build_attn()
    return _CACHED["nc"]


def _make_in_maps(inputs):
    x = np.asarray(inputs["x"], np.float32)
    W_q = np.asarray(inputs["W_q"], np.float32)
    W_k = np.asarray(inputs["W_k"], np.float32)
    W_v = np.asarray(inputs["W_v"], np.float32)
    W_o = np.asarray(inputs["W_o"], np.float32)

    scale = np.float32(1.0 / math.sqrt(D))
    wq_s = np.ascontiguousarray(W_q * scale).astype(BFNP)
    wk_c = np.ascontiguousarray(W_k).astype(BFNP)
    wv_c = np.ascontiguousarray(W_v).astype(BFNP)
    wo_c = np.ascontiguousarray(W_o).astype(BFNP)

    toks = x.reshape(B * S, D)              # (4096, 2048)
    xt_full = np.ascontiguousarray(toks.T)  # (2048, 4096)

    in_maps = []
    for c in range(NCORES):
        xt_c = xt_full[:, c * TOK:(c + 1) * TOK]          # (2048, 512)
        # SBUF layout: (128 partitions, 16*512) with block dt = rows of
        # x^T[dt*128:(dt+1)*128, :]
        xt_r = np.ascontiguousarray(
            xt_c.reshape(NT, P, TOK).transpose(1, 0, 2).reshape(P, NT * TOK))
        in_maps.append({
            "xt": xt_r.astype(BFNP),
            "wq": wq_s, "wk": wk_c, "wv": wv_c, "wo": wo_c,
        })
    return in_maps


def kernel(x, W_q, W_k, W_v, W_o):
    in_maps = _make_in_maps(dict(x=x, W_q=W_q, W_k=W_k, W_v=W_v, W_o=W_o))
    nc = _get_nc()
    res = run_bass_kernel_spmd(nc, in_maps, core_ids=list(range(NCORES)))
    rows = np.concatenate([res.results[c]["out"] for c in range(NCORES)], axis=0)
    return rows.reshape(B, S, D)

